# revision 1
# baseline (speedup 1.0000x reference)
"""Correlation kernel (FlowNet-style, W-displacement only) for Trainium2.

out[b, j, h, w] = mean_c f1[b,c,h,w] * f2pad[b,c,h,w+j],  j in [0, 81), pad=40.

Sharding: data-parallel over batch B=8 across 8 cores (1 batch elem/core).

Device-side work per core (per h row):
  1. Convert f1/f2 chunks fp32 -> fp16 (plain contiguous copies, ACT/DVE).
  2. 3 fp16 matmuls (contraction over C=128 on partitions) produce Gram tiles
     G[w', u] = sum_c f1[c, w0+w'] * f2[c, u0+u] in PSUM (fp32).
  3. ACT/DVE/Pool copy Gram cols PSUM -> SBUF fp16, packed 480 cols/row.
  4. One chunked DMA per 8 rows dumps the packed Gram tiles to DRAM (fp16).

No f2 zero-padding on device: each block's rhs window is clamped to the valid
[0, W) range and the host zero-pads the out-of-range displacements. The
diagonal band extraction (out[j,w] = G[w, w+j-40]) is a shear, which no
on-chip engine can address (per-partition offsets are illegal); the host does
it for free with numpy as_strided during the unshard step. Per-core DRAM
traffic is ~43MB (31.5MB in + 11.8MB out) vs ~84MB for a bounce-based kernel.

DMA queues: input loads ride the SP (sync) ring, dumps ride the ACT (scalar)
ring, so a dump waiting on compute never head-of-line blocks the next chunk's
loads.
"""

import numpy as np
from contextlib import ExitStack

B, C, H, W = 8, 128, 96, 320
D = 40
J = 2 * D + 1  # 81
N_CORES = 8

HCHUNK = 8
NCHUNK = H // HCHUNK
WB = [0, 128, 256]     # w-block starts (lhsT = f1 cols [w0, w0+M))
WN = [128, 128, 128]   # lhsT widths (block 2 spans 64 slack cols)
US = [0, 88, 216]      # rhs window starts (clamped to [0, W))
CPB = [168, 208, 104]  # Gram cols per block (= clamped band cover)
COFF = [0, 168, 376]   # col offsets in the packed dump row
DUMPW = 480
SLACK = 64             # f1h slack so block-2 lhsT can be 128 wide


def _build():
    from concourse import bacc, mybir
    import concourse.tile as tile

    f32 = mybir.dt.float32
    f16 = mybir.dt.float16
    nc = bacc.Bacc(
        "TRN2",
        target_bir_lowering=False,
        debug=False,
        enable_asserts=False,
        num_devices=N_CORES,
    )
    f1 = nc.dram_tensor("f1", [C, H, W], f32, kind="ExternalInput").ap()
    f2 = nc.dram_tensor("f2", [C, H, W], f32, kind="ExternalInput").ap()
    outa = nc.dram_tensor("outa", [128, H, COFF[2]], f16, kind="ExternalOutput").ap()
    outb = nc.dram_tensor("outb", [64, H, CPB[2]], f16, kind="ExternalOutput").ap()

    with tile.TileContext(nc) as tc, ExitStack() as ctx:
        f1r_pool = ctx.enter_context(tc.tile_pool(name="f1r", bufs=3))
        f2r_pool = ctx.enter_context(tc.tile_pool(name="f2r", bufs=3))
        f1h_pool = ctx.enter_context(tc.tile_pool(name="f1h", bufs=2))
        f2h_pool = ctx.enter_context(tc.tile_pool(name="f2h", bufs=2))
        g_pool = ctx.enter_context(tc.tile_pool(name="gsb", bufs=3))
        gb_pool = ctx.enter_context(tc.tile_pool(name="gbsb", bufs=3))
        ps01_pool = ctx.enter_context(tc.tile_pool(name="ps01", bufs=5, space="PSUM"))
        ps2_pool = ctx.enter_context(tc.tile_pool(name="ps2", bufs=3, space="PSUM"))

        chunks = [(0, 4), (4, 4)]
        chunks += [(8 + i * HCHUNK, HCHUNK) for i in range(NCHUNK - 2)]
        chunks += [((NCHUNK - 1) * HCHUNK, 4), ((NCHUNK - 1) * HCHUNK + 4, 4)]
        for ci, (h0, hc) in enumerate(chunks):
            # fill/drain chunks skip the slow Pool conversion slice so the
            # first matmuls start earlier and the last dumps drain sooner
            edge = ci < 2 or ci >= len(chunks) - 3
            f1r = f1r_pool.tile([C, hc * W], f32)
            nc.sync.dma_start(f1r[:], f1[:, h0 : h0 + hc, :])
            f2r = f2r_pool.tile([C, hc * W], f32)
            nc.sync.dma_start(f2r[:], f2[:, h0 : h0 + hc, :])

            # conversions split across ACT / DVE / Pool by measured rates
            nh = hc * W
            f1s = f1h_pool.tile([C, nh + SLACK], f16)
            nc.scalar.copy(f1s[:, 0:nh], f1r[:, 0:nh])
            nc.gpsimd.memset(f1s[:, nh :], 0.0)
            f2s = f2h_pool.tile([C, nh], f16)
            if edge:
                na = (nh * 9) // 20
                nc.scalar.copy(f2s[:, 0:na], f2r[:, 0:na])
                nc.vector.tensor_copy(f2s[:, na:nh], f2r[:, na:nh])
            else:
                na, nd = nh // 5, nh // 2
                nc.scalar.copy(f2s[:, 0:na], f2r[:, 0:na])
                nc.vector.tensor_copy(f2s[:, na:nd], f2r[:, na:nd])
                nc.gpsimd.tensor_copy(f2s[:, nd:nh], f2r[:, nd:nh])

            ga = g_pool.tile([C, hc * COFF[2]], f16, tag="ga")
            gb = gb_pool.tile([64, hc * CPB[2]], f16, tag="gb")
            for h4 in range(0, hc, 4):
                # block-2 Gram tiles for 4 rows share one PSUM bank
                p2 = ps2_pool.tile([128, 4 * CPB[2]], f32, tag="p2")
                for dh in range(4):
                    h = h4 + dh
                    base = h * W
                    # blocks 0+1 share one PSUM bank tile [128, 376]
                    p01 = ps01_pool.tile([128, CPB[0] + CPB[1]], f32, tag="p01")
                    for bi in (0, 1):
                        nc.tensor.matmul(
                            p01[:, COFF[bi] : COFF[bi] + CPB[bi]],
                            lhsT=f1s[:, base + WB[bi] : base + WB[bi] + WN[bi]],
                            rhs=f2s[:, base + US[bi] : base + US[bi] + CPB[bi]],
                            start=True,
                            stop=True,
                        )
                    nc.tensor.matmul(
                        p2[:, dh * CPB[2] : (dh + 1) * CPB[2]],
                        lhsT=f1s[:, base + WB[2] : base + WB[2] + WN[2]],
                        rhs=f2s[:, base + US[2] : base + US[2] + CPB[2]],
                        start=True,
                        stop=True,
                    )
                    # blocks 0+1 -> fp16 staging in one DVE copy
                    nc.vector.tensor_copy(
                        ga[:, h * COFF[2] : (h + 1) * COFF[2]], p01[:]
                    )
                # block-2 of 4 rows -> dense staging in one ACT copy
                nc.scalar.copy(
                    gb[:, h4 * CPB[2] : (h4 + 4) * CPB[2]], p2[0:64, :]
                )
                # dump this 4-row group on the ACT DMA ring
                nc.scalar.dma_start(
                    outa[:, h0 + h4 : h0 + h4 + 4, :],
                    ga[:, h4 * COFF[2] : (h4 + 4) * COFF[2]].rearrange(
                        "p (h c) -> p h c", h=4
                    ),
                )
            nc.scalar.dma_start(
                outb[:, h0 : h0 + hc, :],
                gb[:].rearrange("p (h c) -> p h c", h=hc),
            )

    nc.finalize()
    return nc


def _run(nc, in_maps, **kwargs):
    from concourse.bass_utils import run_bass_kernel_spmd

    return run_bass_kernel_spmd(nc, in_maps, core_ids=list(range(N_CORES)), **kwargs)


def _assemble(dumps_a, dumps_b):
    """dumps_a: [128, H, 376] fp16 per core (blocks 0+1); dumps_b: [64, H, 104]
    fp16 per core (block 2).

    Block bi covers w = WB[bi]+w'; its dump cols hold G[w, US[bi]+c];
    out[b,j,h,w] = G[w, w+j-40]/C with zeros where w+j-40 is outside [0, W).
    """
    ga = np.stack(dumps_a, axis=0)  # [B, 128, H, 376]
    gbk = np.stack(dumps_b, axis=0)  # [B, 64, H, 104]
    out = np.empty((B, J, H, W), dtype=np.float32)
    z40 = lambda shp: np.zeros(shp, dtype=np.float16)
    for bi in range(3):
        wn = min(WN[bi], W - WB[bi])
        if bi < 2:
            blk = ga[:, :wn, :, COFF[bi] : COFF[bi] + CPB[bi]]
        else:
            blk = gbk
        if bi == 0:
            blk = np.concatenate([z40(blk.shape[:3] + (40,)), blk], axis=3)
        elif bi == 2:
            blk = np.concatenate([blk, z40(blk.shape[:3] + (40,))], axis=3)
        blk = np.ascontiguousarray(blk)
        sb, sw, sh, sc = blk.strides
        band = np.lib.stride_tricks.as_strided(
            blk, shape=(B, H, wn, J), strides=(sb, sh, sw + sc, sc)
        )
        # band[b, h, w', j] -> out[b, j, h, w0+w']
        out[:, :, :, WB[bi] : WB[bi] + wn] = band.transpose(0, 3, 1, 2)
    out *= 1.0 / C
    return out


def kernel(f1: np.ndarray, f2: np.ndarray, **run_kwargs) -> np.ndarray:
    assert f1.shape == (B, C, H, W) and f2.shape == (B, C, H, W)
    nc = _build()
    in_maps = [
        {
            "f1": np.ascontiguousarray(f1[i], dtype=np.float32),
            "f2": np.ascontiguousarray(f2[i], dtype=np.float32),
        }
        for i in range(N_CORES)
    ]
    res = _run(nc, in_maps, **run_kwargs)
    out = _assemble(
        [r["outa"] for r in res.results], [r["outb"] for r in res.results]
    )
    if run_kwargs:
        kernel.last_results = res
    return out



# revision 5
# speedup vs baseline: 1.6595x; 1.6595x over previous
"""Correlation kernel (FlowNet-style, W-displacement only) for Trainium2.

out[b, j, h, w] = mean_c f1[b,c,h,w] * f2pad[b,c,h,w+j],  j in [0, 81), pad=40.

Sharding: data-parallel over batch B=8 across 8 cores (1 batch elem/core).

The kernel is HBM-bandwidth bound (358 GB/s/core), so the design minimizes
DRAM traffic:
  * inputs are cast to fp16 on the HOST and uploaded as fp16 (15.7 MB/core
    instead of 31.5 MB fp32) -- no on-device conversion work at all.
  * the Gram band is computed in 64-wide w-groups: for each h row and group
    G (w in [64G, 64G+64)), a single M=64 matmul contracts C=128 against a
    per-group 144-col (104 at the edges) window of f2, so PSUM holds exactly
    the clamped correlation band with no rectangle waste beyond the window.
    Two groups share each PSUM bank via col-tiling (tile_position 0/64), and
    the G3/G4 tiles stack two adjacent h rows so every dumped byte is used.
  * the band is drained PSUM->SBUF as fp16 (DVE/ACT/Pool) and dumped as
    7.86 MB/core. Host does the final diagonal shear with as_strided (free).

Per-row group layout (w = 64G + i, i in [0,64), j in [0,81)):
  G0: rhs window u=[0,104)    c = i+j-40  (c<0   -> zero pad on host)
  G1: rhs window u=[24,168)   c = i+j
  G2: rhs window u=[88,232)   c = i+j
  G3: rhs window u=[152,296)  c = i+j
  G4: rhs window u=[216,320)  c = i+j    (c>=104 -> zero pad on host)

PSUM packing per 4-row group (rows r0..r3):
  bankA [128,496]: r0{ [0:64,0:104]=G0 [64:128,0:104]=G4 [0:64,104:248]=G1
                       [64:128,104:248]=G2 }  r1 same at cols 248:496
  bankB [128,496]: r2, r3 likewise
  bankC [128,288]: [0:64,0:144]=G3_r0 [64:128,0:144]=G3_r1
                   [0:64,144:288]=G3_r2 [64:128,144:288]=G3_r3

Dump: outd[128, 24, 1280] fp16; 4-row group t occupies [:, t, :] with
cols 0:496=bankA, 496:992=bankB, 992:1280=bankC.

DMA queues: input loads ride the SP (sync) ring, dumps ride the ACT (scalar)
ring, so a dump waiting on compute never head-of-line blocks the next chunk's
loads.
"""

import numpy as np
from contextlib import ExitStack

B, C, H, W = 8, 128, 96, 320
D = 40
J = 2 * D + 1  # 81
N_CORES = 8

HCHUNK = 8                  # rows per load chunk / dump
NCHUNK = H // HCHUNK        # 12
NG4 = H // 4                # 24 four-row groups
GCOLS = 1280                # dump cols per 4-row group (496+496+288)
# per-group rhs window starts and widths
GS = [0, 24, 88, 152, 216]
GW = [104, 144, 144, 144, 104]


def _build():
    from concourse import bacc, mybir
    import concourse.tile as tile

    f32 = mybir.dt.float32
    f16 = mybir.dt.float16
    nc = bacc.Bacc(
        "TRN2",
        target_bir_lowering=False,
        debug=False,
        enable_asserts=False,
        num_devices=N_CORES,
    )
    f1 = nc.dram_tensor("f1", [C, H, W], f16, kind="ExternalInput").ap()
    f2 = nc.dram_tensor("f2", [C, H, W], f16, kind="ExternalInput").ap()
    outd = nc.dram_tensor("outd", [128, NG4, GCOLS], f16, kind="ExternalOutput").ap()

    with tile.TileContext(nc) as tc, ExitStack() as ctx:
        f1r_pool = ctx.enter_context(tc.tile_pool(name="f1r", bufs=3))
        f2r_pool = ctx.enter_context(tc.tile_pool(name="f2r", bufs=3))
        stage_pool = ctx.enter_context(tc.tile_pool(name="stg", bufs=3))
        psa_pool = ctx.enter_context(tc.tile_pool(name="psa", bufs=3, space="PSUM"))
        psb_pool = ctx.enter_context(tc.tile_pool(name="psb", bufs=3, space="PSUM"))
        psy_pool = ctx.enter_context(tc.tile_pool(name="psy", bufs=2, space="PSUM"))

        for ci in range(NCHUNK):
            h0 = ci * HCHUNK
            f1r = f1r_pool.tile([C, HCHUNK * W], f16)
            nc.sync.dma_start(f1r[:], f1[:, h0 : h0 + HCHUNK, :])
            f2r = f2r_pool.tile([C, HCHUNK * W], f16)
            nc.sync.dma_start(f2r[:], f2[:, h0 : h0 + HCHUNK, :])

            S = stage_pool.tile([128, 2 * GCOLS], f16)
            for g2 in range(2):  # two 4-row groups per chunk
                pA = psa_pool.tile([128, 496], f32, tag="psA")
                pB = psb_pool.tile([128, 496], f32, tag="psB")
                pC = psy_pool.tile([128, 288], f32, tag="psC")
                for d in range(4):
                    rb = (g2 * 4 + d) * W
                    bank = pA if d < 2 else pB
                    off = (d % 2) * 248
                    # G0 / G4 share cols off:off+104 via col-tiling
                    nc.tensor.matmul(
                        bank[0:64, off : off + 104],
                        lhsT=f1r[:, rb : rb + 64],
                        rhs=f2r[:, rb : rb + 104],
                        start=True, stop=True,
                    )
                    nc.tensor.matmul(
                        bank[64:128, off : off + 104],
                        lhsT=f1r[:, rb + 256 : rb + 320],
                        rhs=f2r[:, rb + 216 : rb + 320],
                        start=True, stop=True,
                    )
                    # G1 / G2 share cols off+104:off+248
                    nc.tensor.matmul(
                        bank[0:64, off + 104 : off + 248],
                        lhsT=f1r[:, rb + 64 : rb + 128],
                        rhs=f2r[:, rb + 24 : rb + 168],
                        start=True, stop=True,
                    )
                    nc.tensor.matmul(
                        bank[64:128, off + 104 : off + 248],
                        lhsT=f1r[:, rb + 128 : rb + 192],
                        rhs=f2r[:, rb + 88 : rb + 232],
                        start=True, stop=True,
                    )
                    # G3 stacks rows pairwise in pC
                    nc.tensor.matmul(
                        pC[64 * (d % 2) : 64 * (d % 2) + 64,
                           144 * (d // 2) : 144 * (d // 2) + 144],
                        lhsT=f1r[:, rb + 192 : rb + 256],
                        rhs=f2r[:, rb + 152 : rb + 296],
                        start=True, stop=True,
                    )
                so = g2 * GCOLS
                nc.vector.tensor_copy(S[:, so : so + 496], pA[:])
                nc.scalar.copy(S[:, so + 496 : so + 992], pB[:])
                if (ci + g2) % 2 == 0:
                    nc.vector.tensor_copy(S[:, so + 992 : so + 1280], pC[:])
                else:
                    nc.scalar.copy(S[:, so + 992 : so + 1280], pC[:])
            nc.scalar.dma_start(
                outd[:, 2 * ci : 2 * ci + 2, :],
                S.rearrange("p (g c) -> p g c", g=2),
            )

    nc.finalize()
    return nc


def _run(nc, in_maps, **kwargs):
    from concourse.bass_utils import run_bass_kernel_spmd

    return run_bass_kernel_spmd(nc, in_maps, core_ids=list(range(N_CORES)), **kwargs)


def _assemble(dumps):
    """dumps: list of B arrays [128, 24, 1280] fp16.

    Recover g[G][b, h, i, c] then band-extract out[b,j,h,64G+i] =
    g[G][b,h,i,i+j(+pad)] / C with as_strided.
    """
    ga = np.stack(dumps, axis=0)  # [B, 128, 24, 1280]
    out = np.empty((B, J, H, W), dtype=np.float32)
    for G in range(5):
        wd = GW[G]
        # gather per-row slices into g[b, h, i, c]
        g = np.empty((B, H, 64, 144), dtype=np.float16)
        if G == 0:
            g[:, :, :, :40] = 0
            dst = g[:, :, :, 40:]
        elif G == 4:
            g[:, :, :, 104:] = 0
            dst = g[:, :, :, :104]
        else:
            dst = g
        for r in range(4):
            if G == 3:
                p0 = 64 * (r % 2)
                c0 = 992 + 144 * (r // 2)
            else:
                half = r % 2
                c0 = (r // 2) * 496 + half * 248
                if G in (0, 4):
                    p0 = 0 if G == 0 else 64
                else:  # G1 -> parts 0:64, G2 -> parts 64:128
                    c0 += 104
                    p0 = 0 if G == 1 else 64
            # outd[:, p0:p0+64, t, c0:c0+wd] -> rows 4t+r
            dst[:, r::4] = ga[:, p0 : p0 + 64, :, c0 : c0 + wd].transpose(0, 2, 1, 3)
        g = np.ascontiguousarray(g)
        sb, sh, si, sc = g.strides
        band = np.lib.stride_tricks.as_strided(
            g, shape=(B, H, 64, J), strides=(sb, sh, si + sc, sc)
        )
        out[:, :, :, 64 * G : 64 * G + 64] = band.transpose(0, 3, 1, 2)
    out *= 1.0 / C
    return out


def kernel(f1: np.ndarray, f2: np.ndarray, **run_kwargs) -> np.ndarray:
    assert f1.shape == (B, C, H, W) and f2.shape == (B, C, H, W)
    f1h = np.ascontiguousarray(f1, dtype=np.float16)
    f2h = np.ascontiguousarray(f2, dtype=np.float16)
    nc = _build()
    in_maps = [{"f1": f1h[i], "f2": f2h[i]} for i in range(N_CORES)]
    res = _run(nc, in_maps, **run_kwargs)
    out = _assemble([r["outd"] for r in res.results])
    if run_kwargs:
        kernel.last_results = res
    return out


# revision 7
# speedup vs baseline: 1.6997x; 1.0242x over previous
"""Correlation kernel (FlowNet-style, W-displacement only) for Trainium2.

out[b, j, h, w] = mean_c f1[b,c,h,w] * f2pad[b,c,h,w+j],  j in [0, 81), pad=40.

Sharding: data-parallel over batch B=8 across 8 cores (1 batch elem/core).

The kernel is HBM-bandwidth bound (~358 GB/s/core), so the design minimizes
DRAM traffic:
  * inputs are cast to fp16 on the HOST and uploaded as ONE interleaved
    tensor fin[C, 2, H, W] (15.7 MB/core instead of 31.5 MB fp32), loaded in
    large chunks (one DMA per chunk) -- no on-device conversion work at all.
  * the Gram band is computed in 64-wide w-groups: for each h row and group
    G (w in [64G, 64G+64)), a single M=64 fp16 matmul contracts C=128 against
    a per-group 144-col (104 at the edges) window of f2, so PSUM holds the
    clamped correlation band with minimal rectangle waste. Two groups share
    each PSUM bank via col-tiling (tile_position 0/64); G3 stacks two
    adjacent h rows so every dumped byte is used.
  * the band drains PSUM->SBUF as fp16 (DVE/ACT alternating) and is dumped
    as 7.86 MB/core. Host does the final diagonal shear with as_strided.

Per-row group layout (w = 64G + i, i in [0,64), j in [0,81)):
  G0: rhs window u=[0,104)    c = i+j-40  (c<0   -> zero pad on host)
  G1: rhs window u=[24,168)   c = i+j
  G2: rhs window u=[88,232)   c = i+j
  G3: rhs window u=[152,296)  c = i+j
  G4: rhs window u=[216,320)  c = i+j    (c>=104 -> zero pad on host)

PSUM packing per 2-row group (rows r0, r1):
  pX [128,496]: r0{ [0:64,0:104]=G0 [64:128,0:104]=G4 [0:64,104:248]=G1
                    [64:128,104:248]=G2 }  r1 same at cols 248:496
  pC [128,144]: [0:64]=G3_r0  [64:128]=G3_r1

Dump: outd[128, 48, 640] fp16; 2-row group t occupies [:, t, :] with
cols 0:496=pX, 496:640=pC.

Chunks shrink toward the end (16,...,16,8,4,2,2 rows) so the final serial
load->matmul->drain->dump chain is short. Input loads ride the SP (sync)
DMA ring, dumps ride the ACT (scalar) ring.
"""

import numpy as np
from contextlib import ExitStack

B, C, H, W = 8, 128, 96, 320
D = 40
J = 2 * D + 1  # 81
N_CORES = 8

NG2 = H // 2   # 48 two-row groups
GCOLS = 640    # dump cols per 2-row group (496 + 144)
GS = [0, 24, 88, 152, 216]   # per-group rhs window starts
GW = [104, 144, 144, 144, 104]
CHUNKS = [16, 16, 16, 16, 16, 8, 4, 2, 2]  # rows per load chunk


def _build():
    from concourse import bacc, mybir
    import concourse.tile as tile

    f32 = mybir.dt.float32
    f16 = mybir.dt.float16
    nc = bacc.Bacc(
        "TRN2",
        target_bir_lowering=False,
        debug=False,
        enable_asserts=False,
        num_devices=N_CORES,
    )
    fin = nc.dram_tensor("fin", [C, 2, H, W], f16, kind="ExternalInput").ap()
    outd = nc.dram_tensor("outd", [128, NG2, GCOLS], f16, kind="ExternalOutput").ap()

    with tile.TileContext(nc) as tc, ExitStack() as ctx:
        fr_pool = ctx.enter_context(tc.tile_pool(name="fr", bufs=3))
        stage_pool = ctx.enter_context(tc.tile_pool(name="stg", bufs=3))
        px_pool = ctx.enter_context(tc.tile_pool(name="px", bufs=5, space="PSUM"))
        pc_pool = ctx.enter_context(tc.tile_pool(name="pc", bufs=3, space="PSUM"))

        eng = 0  # alternate drain engines
        h0 = 0
        for hc in CHUNKS:
            fr = fr_pool.tile([C, 2 * hc * W], f16)
            nc.sync.dma_start(fr[:], fin[:, :, h0 : h0 + hc, :])
            f2o = hc * W  # f2 col offset within fr

            S = stage_pool.tile([128, (hc // 2) * GCOLS], f16)
            for g2 in range(hc // 2):  # 2-row groups in this chunk
                pX = px_pool.tile([128, 496], f32, tag="px")
                pC = pc_pool.tile([128, 144], f32, tag="pc")
                for d in range(2):
                    rb = (g2 * 2 + d) * W
                    off = d * 248
                    # G0 / G4 share cols off:off+104 via col-tiling
                    nc.tensor.matmul(
                        pX[0:64, off : off + 104],
                        lhsT=fr[:, rb : rb + 64],
                        rhs=fr[:, f2o + rb : f2o + rb + 104],
                        start=True, stop=True,
                    )
                    nc.tensor.matmul(
                        pX[64:128, off : off + 104],
                        lhsT=fr[:, rb + 256 : rb + 320],
                        rhs=fr[:, f2o + rb + 216 : f2o + rb + 320],
                        start=True, stop=True,
                    )
                    # G1 / G2 share cols off+104:off+248
                    nc.tensor.matmul(
                        pX[0:64, off + 104 : off + 248],
                        lhsT=fr[:, rb + 64 : rb + 128],
                        rhs=fr[:, f2o + rb + 24 : f2o + rb + 168],
                        start=True, stop=True,
                    )
                    nc.tensor.matmul(
                        pX[64:128, off + 104 : off + 248],
                        lhsT=fr[:, rb + 128 : rb + 192],
                        rhs=fr[:, f2o + rb + 88 : f2o + rb + 232],
                        start=True, stop=True,
                    )
                    # G3 stacks the two rows in pC
                    nc.tensor.matmul(
                        pC[64 * d : 64 * d + 64, :],
                        lhsT=fr[:, rb + 192 : rb + 256],
                        rhs=fr[:, f2o + rb + 152 : f2o + rb + 296],
                        start=True, stop=True,
                    )
                so = g2 * GCOLS
                if eng == 0:
                    nc.vector.tensor_copy(S[:, so : so + 496], pX[:])
                    nc.scalar.copy(S[:, so + 496 : so + 640], pC[:])
                else:
                    nc.scalar.copy(S[:, so : so + 496], pX[:])
                    nc.vector.tensor_copy(S[:, so + 496 : so + 640], pC[:])
                eng ^= 1
            t0 = h0 // 2
            nc.scalar.dma_start(
                outd[:, t0 : t0 + hc // 2, :],
                S.rearrange("p (g c) -> p g c", g=hc // 2),
            )
            h0 += hc

    nc.finalize()
    return nc


def _run(nc, in_maps, **kwargs):
    from concourse.bass_utils import run_bass_kernel_spmd

    return run_bass_kernel_spmd(nc, in_maps, core_ids=list(range(N_CORES)), **kwargs)


def _assemble(dumps):
    """dumps: list of B arrays [128, 48, 640] fp16.

    Recover g[G][b, h, i, c] then band-extract out[b,j,h,64G+i] =
    g[G][b,h,i,i+j(+pad)] / C with as_strided.
    """
    ga = np.stack(dumps, axis=0)  # [B, 128, 48, 640]
    out = np.empty((B, J, H, W), dtype=np.float32)
    for G in range(5):
        wd = GW[G]
        g = np.empty((B, H, 64, 144), dtype=np.float16)
        if G == 0:
            g[:, :, :, :40] = 0
            dst = g[:, :, :, 40:]
        elif G == 4:
            g[:, :, :, 104:] = 0
            dst = g[:, :, :, :104]
        else:
            dst = g
        for r in range(2):
            if G == 3:
                p0, c0 = 64 * r, 496
            else:
                c0 = r * 248 + (104 if G in (1, 2) else 0)
                p0 = 0 if G in (0, 1) else 64
            # outd[:, p0:p0+64, t, c0:c0+wd] -> rows 2t+r
            dst[:, r::2] = ga[:, p0 : p0 + 64, :, c0 : c0 + wd].transpose(0, 2, 1, 3)
        g = np.ascontiguousarray(g)
        sb, sh, si, sc = g.strides
        band = np.lib.stride_tricks.as_strided(
            g, shape=(B, H, 64, J), strides=(sb, sh, si + sc, sc)
        )
        out[:, :, :, 64 * G : 64 * G + 64] = band.transpose(0, 3, 1, 2)
    out *= 1.0 / C
    return out


def kernel(f1: np.ndarray, f2: np.ndarray, **run_kwargs) -> np.ndarray:
    assert f1.shape == (B, C, H, W) and f2.shape == (B, C, H, W)
    fin = np.empty((B, C, 2, H, W), dtype=np.float16)
    fin[:, :, 0] = f1
    fin[:, :, 1] = f2
    nc = _build()
    in_maps = [{"fin": fin[i]} for i in range(N_CORES)]
    res = _run(nc, in_maps, **run_kwargs)
    out = _assemble([r["outd"] for r in res.results])
    if run_kwargs:
        kernel.last_results = res
    return out


# revision 10
# speedup vs baseline: 1.7015x; 1.0011x over previous
"""Correlation kernel (FlowNet-style, W-displacement only) for Trainium2.

out[b, j, h, w] = mean_c f1[b,c,h,w] * f2pad[b,c,h,w+j],  j in [0, 81), pad=40.

Sharding: data-parallel over batch B=8 across 8 cores (1 batch elem/core).

The kernel is HBM-bandwidth bound (~358 GB/s/core), so the design minimizes
DRAM traffic:
  * inputs are cast to fp16 on the HOST and uploaded as ONE interleaved
    tensor fin[C, 2, H, W] (15.7 MB/core instead of 31.5 MB fp32), loaded in
    large chunks (one DMA per chunk) -- no on-device conversion work at all.
  * the Gram band is computed in 64-wide w-groups: for each h row and group
    G (w in [64G, 64G+64)), a single M=64 fp16 matmul contracts C=128 against
    a per-group 144-col (104 at the edges) window of f2, so PSUM holds the
    clamped correlation band with minimal rectangle waste. Two groups share
    each PSUM bank via col-tiling (tile_position 0/64); G3 stacks two
    adjacent h rows so every dumped byte is used.
  * the band drains PSUM->SBUF as fp16 (DVE/ACT alternating) and is dumped
    as 7.86 MB/core. Host does the final diagonal shear with as_strided.

Per-row group layout (w = 64G + i, i in [0,64), j in [0,81)):
  G0: rhs window u=[0,104)    c = i+j-40  (c<0   -> zero pad on host)
  G1: rhs window u=[24,168)   c = i+j
  G2: rhs window u=[88,232)   c = i+j
  G3: rhs window u=[152,296)  c = i+j
  G4: rhs window u=[216,320)  c = i+j    (c>=104 -> zero pad on host)

PSUM packing per 2-row group (rows r0, r1):
  pX [128,496]: r0{ [0:64,0:104]=G0 [64:128,0:104]=G4 [0:64,104:248]=G1
                    [64:128,104:248]=G2 }  r1 same at cols 248:496
  pC [128,144]: [0:64]=G3_r0  [64:128]=G3_r1

Dump: outd[128, 48, 640] fp16; 2-row group t occupies [:, t, :] with
cols 0:496=pX, 496:640=pC.

Chunks shrink toward the end (16,...,16,8,4,2,2 rows) so the final serial
load->matmul->drain->dump chain is short. Input loads ride the SP (sync)
DMA ring, dumps ride the ACT (scalar) ring.
"""

import numpy as np
from contextlib import ExitStack

B, C, H, W = 8, 128, 96, 320
D = 40
J = 2 * D + 1  # 81
N_CORES = 8

NG2 = H // 2   # 48 two-row groups
GCOLS = 640    # dump cols per 2-row group (496 + 144)
GS = [0, 24, 88, 152, 216]   # per-group rhs window starts
GW = [104, 144, 144, 144, 104]
# rows per load chunk: small ramp-in so the first matmuls start early,
# big middle chunks for DMA efficiency, small tail for a short drain chain
CHUNKS = [2, 2, 4, 8, 16, 16, 16, 16, 8, 4, 2, 2]


def _build():
    from concourse import bacc, mybir
    import concourse.tile as tile

    f32 = mybir.dt.float32
    f16 = mybir.dt.float16
    nc = bacc.Bacc(
        "TRN2",
        target_bir_lowering=False,
        debug=False,
        enable_asserts=False,
        num_devices=N_CORES,
    )
    fin = nc.dram_tensor("fin", [C, 2, H, W], f16, kind="ExternalInput").ap()
    outd = nc.dram_tensor("outd", [128, NG2, GCOLS], f16, kind="ExternalOutput").ap()

    with tile.TileContext(nc) as tc, ExitStack() as ctx:
        fr_pool = ctx.enter_context(tc.tile_pool(name="fr", bufs=4))
        stage_pool = ctx.enter_context(tc.tile_pool(name="stg", bufs=6))
        px_pool = ctx.enter_context(tc.tile_pool(name="px", bufs=5, space="PSUM"))
        pc_pool = ctx.enter_context(tc.tile_pool(name="pc", bufs=3, space="PSUM"))

        eng = 0  # alternate drain engines
        h0 = 0
        S = None  # staging tile covering 2 groups (4 rows) per dump
        for hc in CHUNKS:
            fr = fr_pool.tile([C, 2 * hc * W], f16)
            nc.sync.dma_start(fr[:], fin[:, :, h0 : h0 + hc, :])
            f2o = hc * W  # f2 col offset within fr

            for g2 in range(hc // 2):  # 2-row groups in this chunk
                pX = px_pool.tile([128, 496], f32, tag="px")
                pC = pc_pool.tile([128, 144], f32, tag="pc")
                for d in range(2):
                    rb = (g2 * 2 + d) * W
                    off = d * 248
                    # G0 / G4 share cols off:off+104 via col-tiling
                    nc.tensor.matmul(
                        pX[0:64, off : off + 104],
                        lhsT=fr[:, rb : rb + 64],
                        rhs=fr[:, f2o + rb : f2o + rb + 104],
                        start=True, stop=True,
                    )
                    nc.tensor.matmul(
                        pX[64:128, off : off + 104],
                        lhsT=fr[:, rb + 256 : rb + 320],
                        rhs=fr[:, f2o + rb + 216 : f2o + rb + 320],
                        start=True, stop=True,
                    )
                    # G1 / G2 share cols off+104:off+248
                    nc.tensor.matmul(
                        pX[0:64, off + 104 : off + 248],
                        lhsT=fr[:, rb + 64 : rb + 128],
                        rhs=fr[:, f2o + rb + 24 : f2o + rb + 168],
                        start=True, stop=True,
                    )
                    nc.tensor.matmul(
                        pX[64:128, off + 104 : off + 248],
                        lhsT=fr[:, rb + 128 : rb + 192],
                        rhs=fr[:, f2o + rb + 88 : f2o + rb + 232],
                        start=True, stop=True,
                    )
                    # G3 stacks the two rows in pC
                    nc.tensor.matmul(
                        pC[64 * d : 64 * d + 64, :],
                        lhsT=fr[:, rb + 192 : rb + 256],
                        rhs=fr[:, f2o + rb + 152 : f2o + rb + 296],
                        start=True, stop=True,
                    )
                if S is None:
                    S = stage_pool.tile([128, 2 * GCOLS], f16)
                    s_fill, s_t0 = 0, h0 // 2 + g2
                so = s_fill * GCOLS
                if eng == 0:
                    nc.vector.tensor_copy(S[:, so : so + 496], pX[:])
                    nc.scalar.copy(S[:, so + 496 : so + 640], pC[:])
                else:
                    nc.scalar.copy(S[:, so : so + 496], pX[:])
                    nc.vector.tensor_copy(S[:, so + 496 : so + 640], pC[:])
                eng ^= 1
                s_fill += 1
                if s_fill == 2:
                    nc.scalar.dma_start(
                        outd[:, s_t0 : s_t0 + 2, :],
                        S.rearrange("p (g c) -> p g c", g=2),
                    )
                    S = None
            h0 += hc
        assert S is None  # 48 groups -> 24 complete dumps

    nc.finalize()
    return nc


def _run(nc, in_maps, **kwargs):
    from concourse.bass_utils import run_bass_kernel_spmd

    return run_bass_kernel_spmd(nc, in_maps, core_ids=list(range(N_CORES)), **kwargs)


def _assemble(dumps):
    """dumps: list of B arrays [128, 48, 640] fp16.

    Recover g[G][b, h, i, c] then band-extract out[b,j,h,64G+i] =
    g[G][b,h,i,i+j(+pad)] / C with as_strided.
    """
    ga = np.stack(dumps, axis=0)  # [B, 128, 48, 640]
    out = np.empty((B, J, H, W), dtype=np.float32)
    for G in range(5):
        wd = GW[G]
        g = np.empty((B, H, 64, 144), dtype=np.float16)
        if G == 0:
            g[:, :, :, :40] = 0
            dst = g[:, :, :, 40:]
        elif G == 4:
            g[:, :, :, 104:] = 0
            dst = g[:, :, :, :104]
        else:
            dst = g
        for r in range(2):
            if G == 3:
                p0, c0 = 64 * r, 496
            else:
                c0 = r * 248 + (104 if G in (1, 2) else 0)
                p0 = 0 if G in (0, 1) else 64
            # outd[:, p0:p0+64, t, c0:c0+wd] -> rows 2t+r
            dst[:, r::2] = ga[:, p0 : p0 + 64, :, c0 : c0 + wd].transpose(0, 2, 1, 3)
        g = np.ascontiguousarray(g)
        sb, sh, si, sc = g.strides
        band = np.lib.stride_tricks.as_strided(
            g, shape=(B, H, 64, J), strides=(sb, sh, si + sc, sc)
        )
        out[:, :, :, 64 * G : 64 * G + 64] = band.transpose(0, 3, 1, 2)
    out *= 1.0 / C
    return out


def kernel(f1: np.ndarray, f2: np.ndarray, **run_kwargs) -> np.ndarray:
    assert f1.shape == (B, C, H, W) and f2.shape == (B, C, H, W)
    fin = np.empty((B, C, 2, H, W), dtype=np.float16)
    fin[:, :, 0] = f1
    fin[:, :, 1] = f2
    nc = _build()
    in_maps = [{"fin": fin[i]} for i in range(N_CORES)]
    res = _run(nc, in_maps, **run_kwargs)
    out = _assemble([r["outd"] for r in res.results])
    if run_kwargs:
        kernel.last_results = res
    return out
